# revision 1
# baseline (speedup 1.0000x reference)
"""Trainium2 Bass kernel for quantized-MoE Bottleneck (nn_Bottleneck_37503654429269).

See previous revisions for the derivation. v3 layout:
- bf16 integer matmuls (quantized activations/weights are small integers).
- Exact jnp.round via +-2^23 fp32 trick.
- Host-side expert routing: (3,1) or (2,2) sample groups per core.
- GN stats via chunked bn_stats + single ones-matmul partition reduce.
- Engine balance: ACT does bn-affines + x-scale + psum drains; Pool (gpsimd)
  does the round-to-bf16 and final relu; DVE does clamps, bn_stats, the
  fused affine_then_add, and the tiny stats math.
"""

import numpy as np

BITS = (2, 4, 8)
EPS = 1e-5
B, C_IN, H, W = 32, 1024, 14, 14
WIDTH, OUTC = 256, 1024
PIX = H * W  # 196
NCORES = 8
RB = float(2.0 ** 23)

_NC_CACHE = {}


# ----------------------------------------------------------------------------
# Device program
# ----------------------------------------------------------------------------

def _build_nc(group_sizes, stage=99):
    from contextlib import ExitStack
    import concourse.bacc as bacc
    import concourse.mybir as mybir
    import concourse.tile as tile

    F32 = mybir.dt.float32
    BF16 = mybir.dt.bfloat16
    ALU = mybir.AluOpType
    ACT = mybir.ActivationFunctionType

    NG = len(group_sizes)
    NS = sum(group_sizes)
    assert NS == 4
    slot0 = [sum(group_sizes[:g]) for g in range(NG)]
    groups = [list(range(slot0[g], slot0[g] + group_sizes[g])) for g in range(NG)]
    chunks = {g: [groups[g][i:i + 2] for i in range(0, len(groups[g]), 2)]
              for g in range(NG)}

    nc = bacc.Bacc("TRN2", target_bir_lowering=False, debug=False,
                   num_devices=NCORES)

    # ---- dram tensors
    # x: [128, 8, 784] partition-major (host pre-transposed)
    x_d = nc.dram_tensor("x", [128, 8, 4 * PIX], F32, kind="ExternalInput")
    w1_d = nc.dram_tensor("w1", [NG, 128, 8, 256], BF16, kind="ExternalInput")
    w2_d = nc.dram_tensor("w2", [NG, 128, 9, 2, 256], BF16, kind="ExternalInput")
    w3_d = nc.dram_tensor("w3", [NG, 128, 2, 1024], BF16, kind="ExternalInput")
    # packed per-partition consts: xs[NG] xb[NG] a1[2NG] b1[2NG] a2[2NG] b2[2NG] gnb[8]
    NCC = NG + NG + 4 * (2 * NG) + 8
    cc_d = nc.dram_tensor("cc", [128, NCC], F32, kind="ExternalInput")
    # gng row + cst row packed: [1, 1024 + 16*NG]
    gr_d = nc.dram_tensor("gr", [1, 1024 + sum(8 * n for n in group_sizes)],
                          F32, kind="ExternalInput")
    out_d = nc.dram_tensor("out", [128, 8, 4 * PIX], F32, kind="ExternalOutput")

    with tile.TileContext(nc) as tc, ExitStack() as ctx:
        res = ctx.enter_context(tc.tile_pool(name="res", bufs=1))
        rot = ctx.enter_context(tc.tile_pool(name="rot", bufs=4))
        mmp = ctx.enter_context(tc.tile_pool(name="mmp", bufs=6, space="PSUM"))
        smp = ctx.enter_context(tc.tile_pool(name="smp", bufs=1, space="PSUM"))

        # ---- loads (order matters for schedule priority)
        CC = res.tile([128, NCC], F32, name="CC", tag="CC")
        nc.sync.dma_start(out=CC, in_=cc_d.ap())
        o = 0
        XS = CC[:, o:o + NG]; o += NG
        XB = CC[:, o:o + NG]; o += NG
        A1 = CC[:, o:o + 2 * NG].rearrange("p (m g) -> p m g", m=2); o += 2 * NG
        B1 = CC[:, o:o + 2 * NG].rearrange("p (m g) -> p m g", m=2); o += 2 * NG
        A2 = CC[:, o:o + 2 * NG].rearrange("p (m g) -> p m g", m=2); o += 2 * NG
        B2 = CC[:, o:o + 2 * NG].rearrange("p (m g) -> p m g", m=2); o += 2 * NG
        GNB = CC[:, o:o + 8]; o += 8

        GR = res.tile([1, 1024 + sum(8 * n for n in group_sizes)], F32,
                      name="GR", tag="GR")
        nc.sync.dma_start(out=GR, in_=gr_d.ap())
        GNG = GR[:, 0:1024]
        CST = GR[:, 1024:]

        W1 = [res.tile([128, 8, 256], BF16, name=f"W1_{g}", tag=f"W1_{g}")
              for g in range(NG)]
        W2 = [res.tile([128, 9, 2, 256], BF16, name=f"W2_{g}", tag=f"W2_{g}")
              for g in range(NG)]
        W3 = [res.tile([128, 2, 1024], BF16, name=f"W3_{g}", tag=f"W3_{g}")
              for g in range(NG)]
        Xh = [res.tile([128, 2, 4 * PIX], F32, name=f"X{h}", tag=f"X{h}")
              for h in range(4)]

        def Xv(kt):
            return Xh[kt // 2][:, kt % 2, :]

        nc.sync.dma_start(out=W1[0], in_=w1_d.ap()[0])
        for h in range(4):
            nc.sync.dma_start(out=Xh[h],
                              in_=x_d.ap()[:, 2 * h:2 * h + 2, :])
        nc.sync.dma_start(out=W2[0], in_=w2_d.ap()[0])
        nc.sync.dma_start(out=W3[0], in_=w3_d.ap()[0])
        for g in range(1, NG):
            nc.sync.dma_start(out=W1[g], in_=w1_d.ap()[g])
            nc.sync.dma_start(out=W2[g], in_=w2_d.ap()[g])
            nc.sync.dma_start(out=W3[g], in_=w3_d.ap()[g])
        ONES = res.tile([128, 1], F32, name="ONES", tag="ONES")
        nc.vector.memset(ONES, 1.0)
        RBT = res.tile([128, 1], F32, name="RBT", tag="RBT")
        nc.vector.memset(RBT, RB)

        # ---------------- x quantization ----------------
        # q0 = clamp(round(x*(lv-1)), 0, lv-1) in bf16
        Xq = [[None] * NG for _ in range(8)]
        for kt in range(8 if stage >= 1 else 0):
            for g in range(NG):
                ns = group_sizes[g]
                xcols = Xv(kt)[:, slot0[g] * PIX:(slot0[g] + ns) * PIX]
                u = rot.tile([128, ns * PIX], F32, name="xu", tag=f"xu{g}")
                nc.scalar.activation(out=u, in_=xcols, func=ACT.Identity,
                                     bias=RBT, scale=XS[:, g:g + 1])
                xq = res.tile([128, ns * PIX], BF16, name=f"Xq{kt}_{g}",
                              tag=f"Xq{kt}_{g}")
                Xq[kt][g] = xq
                nc.vector.tensor_scalar(out=xq, in0=u, scalar1=RB, scalar2=None,
                                        op0=ALU.subtract)
                nc.vector.tensor_scalar(out=xq, in0=xq, scalar1=0.0,
                                        scalar2=XB[:, g:g + 1],
                                        op0=ALU.max, op1=ALU.min)

        # ---------------- conv1 + bn1 + quant ----------------
        HP = [[None] * NG for _ in range(2)]
        for mo in range(2 if stage >= 1 else 0):
            for g in range(NG):
                ns = group_sizes[g]
                hp = res.tile([128, ns, 16, 18], BF16, name=f"HP{mo}_{g}",
                              tag=f"HP{mo}_{g}")
                nc.vector.memset(hp, 0.0)
                HP[mo][g] = hp

        def bn_round(ps_flat, g, mo, A, Bt, nchunk):
            """psum [128, nchunk*PIX] -> rounded integer bf16 (unclamped)."""
            tpr = rot.tile([128, nchunk * PIX], F32, name="tpr", tag="tpr")
            nc.scalar.activation(out=tpr, in_=ps_flat, func=ACT.Identity,
                                 bias=Bt[:, mo, g:g + 1], scale=A[:, mo, g:g + 1])
            rr = rot.tile([128, nchunk * PIX], BF16, name="rr", tag="rr")
            nc.vector.tensor_scalar(out=rr, in0=tpr, scalar1=RB, scalar2=RB,
                                    op0=ALU.add, op1=ALU.subtract)
            return rr

        for g in range(NG if stage >= 1 else 0):
            for mo in range(2):
                for ch in chunks[g]:
                    nchunk = len(ch)
                    c0 = ch[0] - slot0[g]
                    ps = mmp.tile([128, nchunk * PIX], F32, name="c1ps",
                                  tag="mm")
                    for kt in range(8):
                        nc.tensor.matmul(
                            ps,
                            W1[g][:, kt, mo * 128:(mo + 1) * 128],
                            Xq[kt][g][:, c0 * PIX:(c0 + nchunk) * PIX],
                            start=(kt == 0), stop=(kt == 7))
                    rr = bn_round(ps, g, mo, A1, B1, nchunk)
                    nc.vector.tensor_scalar(
                        out=HP[mo][g][:, c0:c0 + nchunk, 1:15, 2:16],
                        in0=rr.rearrange("p (s y x) -> p s y x", s=nchunk, y=14),
                        scalar1=0.0, scalar2=XB[:, g:g + 1],
                        op0=ALU.max, op1=ALU.min)

        # ---------------- conv2 + bn2 + quant ----------------
        Q2 = [[None] * NG for _ in range(2)]
        for mo in range(2):
            for g in range(NG):
                ns = group_sizes[g]
                Q2[mo][g] = res.tile([128, ns * PIX], BF16, name=f"Q2{mo}_{g}",
                                     tag=f"Q2{mo}_{g}")
        for g in range(NG if stage >= 2 else 0):
            for mo in range(2):
                for ch in chunks[g]:
                    nchunk = len(ch)
                    c0 = ch[0] - slot0[g]
                    ps = mmp.tile([128, nchunk, 14, 14], F32, name="c2ps",
                                  tag="mm")
                    first = True
                    for ti, (dy, dx) in enumerate(
                            (dy, dx) for dy in range(3) for dx in range(3)):
                        for kt in range(2):
                            nc.tensor.matmul(
                                ps,
                                W2[g][:, ti, kt, mo * 128:(mo + 1) * 128],
                                HP[kt][g][:, c0:c0 + nchunk,
                                          dy:dy + 14, dx + 1:dx + 15],
                                start=first, stop=(ti == 8 and kt == 1))
                            first = False
                    rr = bn_round(ps.rearrange("p s y x -> p (s y x)"),
                                  g, mo, A2, B2, nchunk)
                    nc.vector.tensor_scalar(
                        out=Q2[mo][g][:, c0 * PIX:(c0 + nchunk) * PIX],
                        in0=rr,
                        scalar1=0.0, scalar2=XB[:, g:g + 1],
                        op0=ALU.max, op1=ALU.min)

        # ---------------- conv3 + GN ----------------
        S3 = [[None] * NG for _ in range(8)]
        BST = [None] * NG
        for g in range(NG):
            ns = group_sizes[g]
            # bn_stats [128, 8*ns, 6] + mean^2 [128, 8*ns, 2] in one tile
            BST[g] = res.tile([128, 8 * ns * 8], F32, name=f"BST{g}",
                              tag=f"BST{g}")
            for mo in range(8):
                S3[mo][g] = res.tile([128, ns * PIX], F32, name=f"S3_{mo}_{g}",
                                     tag=f"S3_{mo}_{g}")

        PQ = [None] * NG

        for g in range(NG if stage >= 3 else 0):
            ns = group_sizes[g]
            nst = 8 * ns
            bstv = BST[g][:, 0:nst * 6].rearrange("p (t c) -> p t c", c=6)
            for mo in range(8):
                for ch in chunks[g]:
                    nchunk = len(ch)
                    c0 = ch[0] - slot0[g]
                    ps = mmp.tile([128, nchunk * PIX], F32, name="c3ps",
                                  tag="mm")
                    for kt in range(2):
                        nc.tensor.matmul(
                            ps,
                            W3[g][:, kt, mo * 128:(mo + 1) * 128],
                            Q2[kt][g][:, c0 * PIX:(c0 + nchunk) * PIX],
                            start=(kt == 0), stop=(kt == 1))
                    nc.scalar.activation(
                        out=S3[mo][g][:, c0 * PIX:(c0 + nchunk) * PIX],
                        in_=ps, func=ACT.Copy, bias=0.0, scale=1.0)
                    for ci in range(nchunk):
                        nc.vector.bn_stats(
                            out=bstv[:, mo * ns + c0 + ci:mo * ns + c0 + ci + 1, :],
                            in_=ps[:, ci * PIX:(ci + 1) * PIX])
            # mean^2 columns (cols 1 and 4 of each 6-tuple)
            mvi = BST[g][:, 0:nst * 6].rearrange(
                "p (t h c) -> p t h c", h=2, c=3)[:, :, :, 1]
            msq = BST[g][:, nst * 6:nst * 8].rearrange("p (t h) -> p t h", h=2)
            nc.vector.tensor_tensor(out=msq, in0=mvi, in1=mvi, op=ALU.mult)

            if stage == 3:
                nc.sync.dma_start(out=out_d.ap()[:, 0, g * 64:g * 64 + 48],
                                  in_=BST[g][:, 0:48])
                continue

            # ---------- partition-reduce + scalar math ----------
            red = smp.tile([1, nst * 8], F32, name="red", tag="red")
            nc.tensor.matmul(red, ONES, BST[g], start=True, stop=True)
            Tg = res.tile([1, nst * 8], F32, name=f"T{g}", tag=f"T{g}")
            nc.scalar.activation(out=Tg, in_=red, func=ACT.Copy,
                                 bias=0.0, scale=1.0)
            # pair-add mo-parity: stats [1,4,2,ns,6] ; msq [1,4,2,ns,2]
            TB = res.tile([1, 4 * ns * 8], F32, name=f"TB{g}", tag=f"TB{g}")
            tv = Tg[:, 0:nst * 6].rearrange("p (m o s c) -> p m o s c",
                                            m=4, o=2, c=6)
            nc.vector.tensor_tensor(
                out=TB[:, 0:4 * ns * 6].rearrange("p (m s c) -> p m s c",
                                                  m=4, c=6),
                in0=tv[:, :, 0, :, :], in1=tv[:, :, 1, :, :], op=ALU.add)
            mv = Tg[:, nst * 6:nst * 8].rearrange("p (m o s c) -> p m o s c",
                                                  m=4, o=2, c=2)
            nc.vector.tensor_tensor(
                out=TB[:, 4 * ns * 6:4 * ns * 8].rearrange(
                    "p (m s c) -> p m s c", m=4, c=2),
                in0=mv[:, :, 0, :, :], in1=mv[:, :, 1, :, :], op=ALU.add)
            # a = sum means ; b = sum M2 ; c = sum mean^2   (each [1, 4*ns])
            tb6 = TB[:, 0:4 * ns * 6].rearrange("p (t c) -> p t c", c=6)
            tb2 = TB[:, 4 * ns * 6:4 * ns * 8].rearrange("p (t c) -> p t c", c=2)
            SC = res.tile([1, 4 * ns * 4], F32, name=f"SC{g}", tag=f"SC{g}")
            scv = SC.rearrange("p (c t) -> p c t", c=4)
            nc.vector.tensor_tensor(out=scv[:, 0, :], in0=tb6[:, :, 1],
                                    in1=tb6[:, :, 4], op=ALU.add)
            nc.vector.tensor_tensor(out=scv[:, 1, :], in0=tb6[:, :, 2],
                                    in1=tb6[:, :, 5], op=ALU.add)
            nc.vector.tensor_tensor(out=scv[:, 2, :], in0=tb2[:, :, 0],
                                    in1=tb2[:, :, 1], op=ALU.add)
            # mean = a/512 ; e2 = (b + 98*c)/50176 ; var = e2 - mean^2
            MEAN = rot.tile([1, 4 * ns], F32, name="MEAN", tag=f"MEAN{g}")
            nc.vector.tensor_scalar(out=MEAN, in0=scv[:, 0, :],
                                    scalar1=1.0 / 512, scalar2=None,
                                    op0=ALU.mult)
            E2 = rot.tile([1, 4 * ns], F32, name="E2", tag=f"E2{g}")
            nc.vector.scalar_tensor_tensor(out=E2, in0=scv[:, 2, :],
                                           scalar=98.0, in1=scv[:, 1, :],
                                           op0=ALU.mult, op1=ALU.add)
            nc.vector.tensor_scalar(out=E2, in0=E2, scalar1=1.0 / (2 * 128 * PIX),
                                    scalar2=None, op0=ALU.mult)
            VAR = rot.tile([1, 4 * ns], F32, name="VAR", tag=f"VAR{g}")
            nc.vector.tensor_tensor(out=VAR, in0=MEAN, in1=MEAN, op=ALU.mult)
            nc.vector.tensor_tensor(out=VAR, in0=E2, in1=VAR, op=ALU.subtract)
            # y = var*c3^2 + eps ; rc = 1/sqrt(y)
            cbase = 1024 + sum(8 * n for n in group_sizes[:g])
            nsc = 4 * ns
            nc.vector.tensor_tensor(out=VAR, in0=VAR,
                                    in1=GR[:, cbase + nsc:cbase + 2 * nsc],
                                    op=ALU.mult)
            nc.vector.tensor_scalar(out=VAR, in0=VAR, scalar1=EPS,
                                    scalar2=None, op0=ALU.add)
            SD = rot.tile([1, 4 * ns], F32, name="SD", tag=f"SD{g}")
            nc.scalar.activation(out=SD, in_=VAR, func=ACT.Sqrt,
                                 bias=0.0, scale=1.0)
            RC = rot.tile([1, 4 * ns], F32, name="RC", tag=f"RC{g}")
            nc.vector.reciprocal(out=RC, in_=SD)
            Fv = res.tile([1, 8 * ns], F32, name=f"F_{g}", tag=f"F_{g}")
            nc.vector.tensor_tensor(out=Fv[:, 0:nsc], in0=RC,
                                    in1=GR[:, cbase:cbase + nsc], op=ALU.mult)
            nc.vector.scalar_tensor_tensor(
                out=Fv[:, nsc:2 * nsc], in0=MEAN, scalar=-1.0,
                in1=Fv[:, 0:nsc], op0=ALU.mult, op1=ALU.mult)
            if stage == 4:
                nc.sync.dma_start(out=out_d.ap()[0:1, 0, g * 64:g * 64 + 8 * ns],
                                  in_=Fv)
                continue
            # P,Q outer products
            pqp = smp.tile([128, 8, 2, ns], F32, name="pqp", tag="pqp")
            fvv = Fv.rearrange("p (k m s) -> p k m s", k=2, m=4)
            for mo in range(8):
                nc.tensor.matmul(
                    pqp[:, mo, :, :],
                    GNG[:, mo * 128:(mo + 1) * 128],
                    fvv[:, :, mo // 2, :],
                    start=(mo == 0), stop=(mo == 7), skip_group_check=True)
            PQ[g] = res.tile([128, 8, 2, ns], F32, name=f"PQ{g}", tag=f"PQ{g}")
            nc.scalar.activation(out=PQ[g], in_=pqp, func=ACT.Copy,
                                 bias=0.0, scale=1.0)

            # ---------- final affine + residual + relu + store ----------
            for mo in range(8):
                V = rot.tile([128, ns * PIX], F32, name="V", tag=f"V{g}")
                for si, slot in enumerate(groups[g]):
                    nc.vector.affine_then_add(
                        out=V[:, si * PIX:(si + 1) * PIX],
                        in0=S3[mo][g][:, si * PIX:(si + 1) * PIX],
                        in1=Xv(mo)[:, slot * PIX:(slot + 1) * PIX],
                        scale=PQ[g][:, mo, 0, si:si + 1],
                        bias=PQ[g][:, mo, 1, si:si + 1])
                ot = rot.tile([128, ns * PIX], F32, name="ot", tag=f"ot{g}")
                nc.vector.tensor_scalar(out=ot, in0=V,
                                        scalar1=GNB[:, mo:mo + 1], scalar2=0.0,
                                        op0=ALU.add, op1=ALU.max)
                nc.sync.dma_start(
                    out=out_d.ap()[:, mo, slot0[g] * PIX:(slot0[g] + ns) * PIX],
                    in_=ot)

    nc.compile()
    return nc


# ----------------------------------------------------------------------------
# Host side
# ----------------------------------------------------------------------------

def _quant_w(w, lv):
    n = max(lv // 2 - 1, 1)
    s = np.float32(np.abs(w).max()) + np.float32(1e-12)
    k = np.round((w.astype(np.float32) / s) * np.float32(n)).astype(np.float32)
    return k, np.float32(s) / np.float32(n)


def _assign_groups(mask):
    mask = np.asarray(mask).astype(np.int64)
    ids = {e: [int(i) for i in np.nonzero(mask == e)[0]] for e in range(3)}
    counts = [len(ids[e]) for e in range(3)]
    if all(c % 2 == 0 for c in counts):
        group_sizes = (2, 2)
        chunks2 = []
        for e in range(3):
            for j in range(0, counts[e], 2):
                chunks2.append((e, ids[e][j:j + 2]))
        assert len(chunks2) == 16
        core_samples = []
        core_experts = []
        for c in range(8):
            (ea, sa), (eb, sb) = chunks2[2 * c], chunks2[2 * c + 1]
            core_samples.append(sa + sb)
            core_experts.append([ea, eb])
        return group_sizes, core_samples, core_experts

    base = [c % 3 for c in counts]
    need = (8 - sum(base)) // 3
    t = [0, 0, 0]
    for e in range(3):
        cap = (counts[e] - base[e]) // 3
        take = min(cap, need)
        t[e] = take
        need -= take
        if need == 0:
            break
    assert need == 0
    b = [base[e] + 3 * t[e] for e in range(3)]
    a = [(counts[e] - b[e]) // 3 for e in range(3)]
    assert sum(a) == 8 and sum(b) == 8
    trip = []
    single = []
    for e in range(3):
        pos = 0
        for _ in range(a[e]):
            trip.append((e, ids[e][pos:pos + 3]))
            pos += 3
        for _ in range(b[e]):
            single.append((e, [ids[e][pos]]))
            pos += 1
        assert pos == counts[e]
    core_samples = []
    core_experts = []
    for c in range(8):
        ea, sa = trip[c]
        eb, sb = single[c]
        core_samples.append(sa + sb)
        core_experts.append([ea, eb])
    return (3, 1), core_samples, core_experts


def kernel(x, mask, w1, w2, w3, bn1_g, bn1_b, bn1_m, bn1_v,
           bn2_g, bn2_b, bn2_m, bn2_v, gn_g, gn_b):
    import os
    import ml_dtypes
    from concourse.bass_utils import run_bass_kernel_spmd

    bf16 = ml_dtypes.bfloat16
    f32 = np.float32
    x = np.asarray(x, f32)
    mask = np.asarray(mask)
    w1 = np.asarray(w1, f32)
    w2 = np.asarray(w2, f32)
    w3 = np.asarray(w3, f32)
    bn1 = [np.asarray(v, f32) for v in (bn1_g, bn1_b, bn1_m, bn1_v)]
    bn2 = [np.asarray(v, f32) for v in (bn2_g, bn2_b, bn2_m, bn2_v)]
    gn_g = np.asarray(gn_g, f32)
    gn_b = np.asarray(gn_b, f32)

    group_sizes, core_samples, core_experts = _assign_groups(mask)
    NG = len(group_sizes)

    lv_of = [2 ** b for b in BITS]
    K1, K2, K3 = {}, {}, {}
    CW = {}
    for e in range(3):
        lv = lv_of[e]
        k1, c1 = _quant_w(w1, lv)
        k2, c2 = _quant_w(w2, lv)
        k3, c3 = _quant_w(w3, lv)
        K1[e] = k1.reshape(256, 1024)
        K2[e] = k2.reshape(256, 256, 3, 3)
        K3[e] = k3.reshape(1024, 256)
        CW[e] = (c1, c2, c3)

    inv1 = bn1[0] / np.sqrt(bn1[3] + f32(EPS))
    bb1 = bn1[1] - bn1[2] * inv1
    inv2 = bn2[0] / np.sqrt(bn2[3] + f32(EPS))
    bb2 = bn2[1] - bn2[2] * inv2

    def pack_w(e):
        k1t = K1[e].T.reshape(8, 128, 256).transpose(1, 0, 2)
        k2t = K2[e].transpose(2, 3, 1, 0).reshape(9, 2, 128, 256)
        k2t = k2t.transpose(2, 0, 1, 3)
        k3t = K3[e].T.reshape(2, 128, 1024).transpose(1, 0, 2)
        return (np.ascontiguousarray(k1t).astype(bf16),
                np.ascontiguousarray(k2t).astype(bf16),
                np.ascontiguousarray(k3t).astype(bf16))

    packed = {e: pack_w(e) for e in set(int(v) for v in np.asarray(mask))}

    in_maps = []
    for c in range(8):
        sids = core_samples[c]
        experts = core_experts[c]

        # x: [128, 8, 784]: (p, kt, slot*196+pix)
        xc = x[sids].reshape(4, 8, 128, PIX).transpose(2, 1, 0, 3) \
                    .reshape(128, 8, 4 * PIX).copy()

        w1c = np.stack([packed[experts[g]][0] for g in range(NG)])
        w2c = np.stack([packed[experts[g]][1] for g in range(NG)])
        w3c = np.stack([packed[experts[g]][2] for g in range(NG)])

        glv = [lv_of[experts[g]] for g in range(NG)]
        cc = np.zeros((128, NG * 2 + 8 * NG + 8), f32)
        cc[:, 0:NG] = [lv - 1 for lv in glv]          # xs
        cc[:, NG:2 * NG] = [lv - 1 for lv in glv]     # xb
        # a1/b1/a2/b2 layout: [p, m, g] flattened as (m g)
        a1 = np.zeros((128, 2, NG), f32)
        b1 = np.zeros((128, 2, NG), f32)
        a2 = np.zeros((128, 2, NG), f32)
        b2 = np.zeros((128, 2, NG), f32)
        for g in range(NG):
            e = experts[g]
            lv = glv[g]
            c1, c2, c3 = CW[e]
            a1[:, :, g] = (inv1 * c1).reshape(2, 128).T
            b1[:, :, g] = (bb1 * f32(lv - 1)).reshape(2, 128).T
            a2[:, :, g] = (inv2 * c2).reshape(2, 128).T
            b2[:, :, g] = (bb2 * f32(lv - 1)).reshape(2, 128).T
        o = 2 * NG
        cc[:, o:o + 2 * NG] = a1.reshape(128, 2 * NG); o += 2 * NG
        cc[:, o:o + 2 * NG] = b1.reshape(128, 2 * NG); o += 2 * NG
        cc[:, o:o + 2 * NG] = a2.reshape(128, 2 * NG); o += 2 * NG
        cc[:, o:o + 2 * NG] = b2.reshape(128, 2 * NG); o += 2 * NG
        cc[:, o:o + 8] = gn_b.reshape(8, 128).T; o += 8

        gr = np.zeros((1, 1024 + sum(8 * n for n in group_sizes)), f32)
        gr[0, 0:1024] = gn_g
        off = 1024
        for g in range(NG):
            ns = group_sizes[g]
            e = experts[g]
            lv = glv[g]
            c3e = CW[e][2] / f32(lv - 1)
            gr[0, off:off + 4 * ns] = c3e
            gr[0, off + 4 * ns:off + 8 * ns] = c3e * c3e
            off += 8 * ns

        in_maps.append({
            "x": xc, "w1": w1c, "w2": w2c, "w3": w3c, "cc": cc, "gr": gr,
        })

    stage = int(os.environ.get("KERNEL_STAGE", "99"))
    key = (group_sizes, stage)
    if key not in _NC_CACHE:
        _NC_CACHE[key] = _build_nc(group_sizes, stage)
    nc = _NC_CACHE[key]

    res = run_bass_kernel_spmd(nc, in_maps, core_ids=list(range(NCORES)))

    out = np.zeros((B, OUTC, H, W), f32)
    for c in range(8):
        oc = res.results[c]["out"]  # [128, 8, 4*PIX]
        oc = oc.reshape(128, 8, 4, PIX).transpose(2, 1, 0, 3) \
               .reshape(4, OUTC, H, W)
        for t, sid in enumerate(core_samples[c]):
            out[sid] = oc[t]
    return out

